# revision 1
# baseline (speedup 1.0000x reference)
"""ExplaiNN Trainium2 kernel — 8-core SPMD, batch-sharded (32 rows/core).

Pipeline per core (all BN affines folded into weights on host):
  conv:  X-stationary strided-position matmuls (fp32r), 7 stride-7 sub-convs
         per row land in PSUM banks so maxpool output is [p, u] (FC1-ready).
  pool:  DVE reduce_max over 5 bank-strided PSUM j-slices + 2 tensor_max ops
         on the remaining 2 slices (5+2 bank split lets PE/DVE ping-pong
         within the 8 PSUM banks).
  exp:   ACT Exp -> pexp bf16 (BN1 folded into conv W; exp(t1+s1*b_conv) into
         W1; maxpool commutes with exp by monotonicity).
  FC1:   per-unit bf16 matmuls, PE col-tiling packs 4 units (M=32 each) into
         the 128 PSUM partitions; const-1 row 127 of pexp carries the bias;
         K=141 split 128 (resident w1a) + 13 (streamed w1b, PSUM-accumulated).
  relu:  ACT Relu PSUM->SBUF bf16.
  FC2:   DVE mul + reduce_add over f, chunked and interleaved into the FC1
         loop with the +b2/relu/*w_out head ops; partition mix-down (sum over
         4 col-strips) via a small fp32 matmul against a host-built selection
         matrix E, ACT Sigmoid(+b_out), DMA out.
Scheduling: B-window blocks run early off a prepacked xcolb tensor so their
layout-repack DMAs leave the critical path; all weight/const prefetch is
emitted on the gpsimd queue in a deadlock-safe order (repacks before slot-
limited w1b groups); slabs stream per-row on the sync queue.
"""

import numpy as np
from contextlib import ExitStack

import concourse.bass as bass
import concourse.bacc as bacc
import concourse.mybir as mybir
import concourse.tile as tile
from concourse.bass_utils import run_bass_kernel_spmd

dt = mybir.dt

U, K, POOL, STRIDE, FC = 300, 19, 7, 7, 100
B, L, D = 256, 1000, 4
P = 140                     # pooled positions per row
EPS = 1e-5
NCORES = 8
BS = B // NCORES            # 32 rows per core
KD = K * D                  # 76 contraction
PA = 127                    # windows in the A-chunk (+1 const row = 128)
PB = P - PA                 # 13 windows in the B-chunk
NJ = 75                     # 300 units / 4 col-strips

_COMPILED = None


def _build(stage=3):
    nc = bacc.Bacc("TRN2", target_bir_lowering=False, debug=False,
                   num_devices=NCORES)

    xcol_d = nc.dram_tensor("xcol", [KD, BS, 980], dt.float32r, kind="ExternalInput").ap()
    xcolb_d = nc.dram_tensor("xcolb", [KD, 4, 7, 104], dt.float32r, kind="ExternalInput").ap()
    wc_d = nc.dram_tensor("wc", [KD, U], dt.float32r, kind="ExternalInput").ap()
    w1a_d = nc.dram_tensor("w1a", [128, U, FC], dt.bfloat16, kind="ExternalInput").ap()
    w1b_d = nc.dram_tensor("w1b", [PB, U, FC], dt.bfloat16, kind="ExternalInput").ap()
    w2e_d = nc.dram_tensor("w2e", [128, NJ, FC], dt.bfloat16, kind="ExternalInput").ap()
    b2e_d = nc.dram_tensor("b2e", [128, NJ], dt.float32, kind="ExternalInput").ap()
    wout_d = nc.dram_tensor("woute", [128, NJ], dt.float32, kind="ExternalInput").ap()
    E_d = nc.dram_tensor("Emat", [128, BS], dt.float32, kind="ExternalInput").ap()
    ones_d = nc.dram_tensor("onesrow", [1, BS, U], dt.bfloat16, kind="ExternalInput").ap()
    bout_d = nc.dram_tensor("bout", [1, 1], dt.float32, kind="ExternalInput").ap()
    out_d = nc.dram_tensor("out", [1, BS], dt.float32, kind="ExternalOutput").ap()

    f32, f32r, bf16 = dt.float32, dt.float32r, dt.bfloat16
    AF = mybir.ActivationFunctionType

    with ExitStack() as ctx:
        tc = ctx.enter_context(tile.TileContext(nc))
        consts = ctx.enter_context(tc.tile_pool(name="consts", bufs=1))

        wc = consts.tile([KD, U], f32r)
        nc.gpsimd.dma_start(wc[:], wc_d[:])
        xcolb = consts.tile([KD, 4, 7, 104], f32r)
        nc.gpsimd.dma_start(xcolb[:], xcolb_d[:])
        b2e = consts.tile([128, NJ], f32)
        woute = consts.tile([128, NJ], f32)
        Emat = consts.tile([128, BS], f32)
        bout = consts.tile([1, 1], f32)

        w1a = consts.tile([128, U, FC], bf16)
        pexp = consts.tile([128, BS, U], bf16)
        pexpB = consts.tile([104, 4, U], bf16)      # [(rr,pb), blk, u]
        pexpB2 = consts.tile([PB, BS, U], bf16)     # [pb, r, u]
        hrelu = consts.tile([128, NJ, FC], bf16)

        wcr = wc[:]
        FC2_CHUNKS = [(0, 15), (15, 15), (30, 15), (45, 15), (60, 12), (72, 3)]

        w1pool = ctx.enter_context(tc.tile_pool(name="w1s", bufs=8))
        w2pool = ctx.enter_context(tc.tile_pool(name="w2s", bufs=2))

        # ---------------- phase 1: conv + pool + exp ----------------
        with tc.tile_pool(name="xslab", bufs=4) as xpool, \
             tc.tile_pool(name="convps", bufs=1, space="PSUM") as cpsum, \
             tc.tile_pool(name="convps2", bufs=2, space="PSUM") as cpsum2, \
             tc.tile_pool(name="pools", bufs=3) as spool:

            def conv_pool_exp(m, lhs, dst_ap, repack_blk=None):
                t5 = cpsum.tile([128, 5, 512], f32, tag="t5")
                t2a = cpsum2.tile([128, 1, 512], f32, tag="t2a")
                t2b = cpsum.tile([128, 1, 512], f32, tag="t2b")
                for j in range(7):
                    dst = (t5[0:m, j, 0:U] if j < 5 else
                           (t2a if j == 5 else t2b)[0:m, 0, 0:U])
                    nc.tensor.matmul(dst, lhs[j], wcr, start=True, stop=True)
                p5 = spool.tile([128, U], f32, tag="p5")
                nc.vector.reduce_max(p5[0:m, :], t5[0:m, :, 0:U].rearrange("p j u -> p u j"),
                                     axis=mybir.AxisListType.X)
                pm1 = spool.tile([128, U], f32, tag="pm1")
                nc.vector.tensor_max(pm1[0:m, :], p5[0:m, :], t2a[0:m, 0, 0:U])
                pm = spool.tile([128, U], f32, tag="pm")
                nc.vector.tensor_max(pm[0:m, :], pm1[0:m, :], t2b[0:m, 0, 0:U])
                nc.scalar.activation(dst_ap, pm[0:m, :], AF.Exp)
                if repack_blk is not None:
                    for _rr in range(8):
                        nc.gpsimd.dma_start(
                            pexpB2[:, 8 * repack_blk + _rr, :],
                            pexpB[_rr * PB:(_rr + 1) * PB, repack_blk, :])

            nc.gpsimd.dma_start(w1a[:], w1a_d[:])

            def a_slab(sl):
                slab = xpool.tile([KD, 2, 980], f32r, tag="slab")
                for _r in range(2):
                    nc.sync.dma_start(slab[:, _r, :], xcol_d[:, sl * 2 + _r, :])
                slabr = slab[:].rearrange("q r (p j) -> q r p j", j=7)
                for rr in range(2):
                    conv_pool_exp(PA, [slabr[:, rr, 0:PA, j] for j in range(7)],
                                  pexp[0:PA, sl * 2 + rr, :])

            # two A-slabs first (slab DMAs on sync start instantly); B-blocks
            # follow once xcolb's gpsimd transfer lands
            for sl in range(2):
                a_slab(sl)
            for blk in range(4):
                conv_pool_exp(8 * PB, [xcolb[:, blk, j, :] for j in range(7)],
                              pexpB[0:8 * PB, blk, :], repack_blk=blk)
            # weight prefetch on gpsimd, emitted after the repacks so slot
            # waits for late w1b groups can never gate pexpB2
            nc.gpsimd.dma_start(pexp[127:128, :, :], ones_d[:])
            nc.gpsimd.dma_start(b2e[:], b2e_d[:])
            nc.gpsimd.dma_start(woute[:], wout_d[:])
            nc.gpsimd.dma_start(Emat[:], E_d[:])
            nc.gpsimd.dma_start(bout[:], bout_d[:])
            w1bs_t = []
            for g in range(19):
                nun = 16 if g < 18 else 12
                w1bs = w1pool.tile([PB, 16, FC], bf16, tag="w1bs")
                nc.gpsimd.dma_start(w1bs[:, 0:nun, :], w1b_d[:, 16 * g:16 * g + nun, :])
                w1bs_t.append(w1bs)
            w2s_t = []
            for c0, cn in FC2_CHUNKS:
                w2s = w2pool.tile([128, 15, FC], bf16, tag="w2s")
                nc.gpsimd.dma_start(w2s[:, 0:cn, :], w2e_d[:, c0:c0 + cn, :])
                w2s_t.append(w2s)
            for sl in range(2, 16):
                a_slab(sl)

        if stage == 1:
            osb1 = consts.tile([1, BS], f32)
            nc.vector.tensor_copy(osb1[0:1, :], pexp[0:1, :, 0])
            nc.sync.dma_start(out_d[:], osb1[:])
            nc.compile2 = True
        # ---------------- phase 2: FC1 + relu ----------------
        z = consts.tile([128, NJ], f32)
        zr = consts.tile([128, NJ], f32)
        ptmp = consts.tile([128, NJ], f32)
        fc2_after = {3: 0, 7: 1, 11: 2, 14: 3, 17: 4, 18: 5}
        if stage >= 2:
          with tc.tile_pool(name="fcps", bufs=2, space="PSUM") as fpsum:
              for g in range(19):
                  nun = 16 if g < 18 else 12
                  w1bs = w1bs_t[g]
                  hps = fpsum.tile([128, 4, 512], f32, tag="hps")
                  for k in range(4):
                      for s in range(nun // 4):
                          uu = 4 * s + k
                          u = 16 * g + uu
                          o = hps[32 * k:32 * k + 32, s, 0:FC]
                          nc.tensor.matmul(o, pexp[:, :, u], w1a[:, u, :],
                                           start=True, stop=False, tile_position=(0, 32 * k))
                          nc.tensor.matmul(o, pexpB2[:, :, u], w1bs[:, uu, :],
                                           start=False, stop=True, tile_position=(0, 32 * k))
                  ns = nun // 4
                  nc.scalar.activation(hrelu[:, 4 * g:4 * g + ns, :], hps[:, 0:ns, 0:FC], AF.Relu)
                  # FC2 chunk as soon as its hrelu columns are complete
                  if stage >= 3 and g in fc2_after:
                      c = fc2_after[g]
                      c0, cn = FC2_CHUNKS[c]
                      slc = slice(c0, c0 + cn)
                      w2s = w2s_t[c]
                      prod = w2pool.tile([128, 15, FC], bf16, tag="prod")
                      nc.vector.tensor_mul(prod[:, 0:cn, :], hrelu[:, slc, :], w2s[:, 0:cn, :])
                      nc.vector.tensor_reduce(z[:, slc], prod[:, 0:cn, :],
                                              axis=mybir.AxisListType.X,
                                              op=mybir.AluOpType.add)
                      nc.vector.tensor_add(zr[:, slc], z[:, slc], b2e[:, slc])
                      nc.vector.tensor_scalar_max(zr[:, slc], zr[:, slc], 0.0)
                      nc.vector.tensor_mul(ptmp[:, slc], zr[:, slc], woute[:, slc])

        if stage == 2:
            osb2 = consts.tile([1, BS], f32)
            nc.vector.tensor_copy(osb2[0:1, :], hrelu[0:1, 0:32, 0])
            nc.sync.dma_start(out_d[:], osb2[:])
        if stage >= 3:
          # ---------------- phase 3: head ----------------
          part = consts.tile([128, 1], f32)
          nc.vector.tensor_reduce(part[:], ptmp[:], axis=mybir.AxisListType.X,
                                  op=mybir.AluOpType.add)
          with tc.tile_pool(name="headps", bufs=1, space="PSUM") as hpsum:
              zf = hpsum.tile([1, BS], f32, tag="zf")
              nc.tensor.matmul(zf[0:1, :], part[:], Emat[:], start=True, stop=True)
              osb = consts.tile([1, BS], f32)
              nc.scalar.activation(osb[:], zf[0:1, :], AF.Sigmoid, bias=bout[0:1, :])
              nc.sync.dma_start(out_d[:], osb[:])

    nc.compile()
    return nc


def _prep_weights(i):
    """Host-side BN folding + layout. All numpy, fp32."""
    f = lambda a: np.asarray(a, np.float32)
    w_conv, b_conv = f(i["w_conv"]), f(i["b_conv"])
    g1, be1, m1, v1 = f(i["g1"]), f(i["be1"]), f(i["m1"]), f(i["v1"])
    w_fc1, b_fc1 = f(i["w_fc1"]), f(i["b_fc1"])
    g2, be2, m2, v2 = f(i["g2"]), f(i["be2"]), f(i["m2"]), f(i["v2"])
    w_fc2, b_fc2 = f(i["w_fc2"]), f(i["b_fc2"])
    g3, be3, m3, v3 = f(i["g3"]), f(i["be3"]), f(i["m3"]), f(i["v3"])
    w_out, b_out = f(i["w_out"]), f(i["b_out"])

    s1 = g1 / np.sqrt(v1 + EPS)
    t1 = be1 - m1 * s1
    s2 = g2 / np.sqrt(v2 + EPS)
    b1pp = (b_fc1 - m2) * s2 + be2
    s3 = g3 / np.sqrt(v3 + EPS)
    w2pp = w_fc2 * s3[:, None]
    b2pp = (b_fc2 - m3) * s3 + be3

    # conv weights, BN1 scale folded; q = k*4 + d
    Wc = np.ascontiguousarray(
        (w_conv * s1[:, None, None]).transpose(2, 1, 0).reshape(KD, U))
    # FC1 with BN2 scale and exp(t1 + s1*b_conv) folded
    gexp = np.exp(t1 + s1 * b_conv)
    w1pp = (w_fc1 * s2[:, :, None] * gexp[:, None, None]).transpose(2, 0, 1)  # (P,U,FC)
    w1a = np.empty((128, U, FC), np.float32)
    w1a[:PA] = w1pp[:PA]
    w1a[127] = b1pp
    w1b = np.ascontiguousarray(w1pp[PA:P])

    js = 4 * np.arange(NJ)
    w2e = np.zeros((128, NJ, FC), np.float32)
    b2e = np.zeros((128, NJ), np.float32)
    woute = np.zeros((128, NJ), np.float32)
    for k in range(4):
        w2e[k * 32:(k + 1) * 32] = w2pp[js + k][None]
        b2e[k * 32:(k + 1) * 32] = b2pp[js + k][None]
        woute[k * 32:(k + 1) * 32] = w_out[js + k, 0][None]
    Em = np.zeros((128, BS), np.float32)
    for k in range(4):
        Em[k * 32:(k + 1) * 32] = np.eye(BS, dtype=np.float32)

    import ml_dtypes
    b16 = lambda a: np.asarray(a, ml_dtypes.bfloat16)
    return {
        "wc": Wc, "w1a": b16(w1a), "w1b": b16(w1b), "w2e": b16(w2e),
        "b2e": b2e, "woute": woute, "Emat": Em,
        "onesrow": np.ones((1, BS, U), ml_dtypes.bfloat16),
        "bout": np.asarray(b_out, np.float32).reshape(1, 1),
    }


def kernel(**inputs) -> np.ndarray:
    global _COMPILED
    if _COMPILED is None:
        _COMPILED = _build()
    nc = _COMPILED

    wmap = _prep_weights(inputs)
    x = np.asarray(inputs["input_seq"], np.float32)   # (256, 1000, 4)
    win = np.lib.stride_tricks.sliding_window_view(x, K, axis=1)  # (B, 982, D, K)
    in_maps = []
    for c in range(NCORES):
        xs = win[c * BS:(c + 1) * BS, :980]           # (32, 980, 4, 19)
        xcol = np.ascontiguousarray(xs.transpose(3, 2, 0, 1)).reshape(KD, BS, 980)
        tail = xcol[:, :, 7 * PA:].reshape(KD, 4, 8, PB, 7)
        xcolb = np.ascontiguousarray(tail.transpose(0, 1, 4, 2, 3)).reshape(KD, 4, 7, 104)
        in_maps.append({"xcol": xcol, "xcolb": xcolb, **wmap})

    res = run_bass_kernel_spmd(nc, in_maps, list(range(NCORES)))
    out = np.empty((B, 1), np.float32)
    for c in range(NCORES):
        out[c * BS:(c + 1) * BS, 0] = res.results[c]["out"][0]
    return out

